# revision 1
# baseline (speedup 1.0000x reference)
"""Multi-head attention (B=8, N=1024, DIM=1152, H=16, hd=72) on 8 TRN2 cores.

Sharding: pure data parallelism -- core i computes batch element i, weights
are replicated. No collectives.

Device-side strategy (per core):
  - x arrives bf16 (host cast); x^T is built by DMA-transpose during load
    (no PE/DVE cost).
  - Q^T, K^T computed in transposed layout [outdim, token] with bf16
    matmuls, stacked compactly in 128-row tiles (qkt).
  - Each head's 72-row Q^T/K^T slice is repacked to partition 0 by
    SBUF->SBUF DMAs (DMA shifts partitions freely; matmul operands must
    start at partition 0/32/64).
  - S^T = K_h @ Q_h^T puts softmax's k-reduction on PSUM partitions; the
    denominator is recovered free via a ones column appended to V
    (AV matmul emits [72+1, q], row 72 = sum_k exp).
  - exp on ScalarE over paired 1024-wide tiles with the 1/sqrt(hd) scale
    folded in; no max subtraction (scores are ~N(0,1), no overflow risk).
  - Normalization: denominator quadrant copied to SBUF, stream_shuffled to
    quadrant 0 (custom DVE ops require base partition 0),
    reciprocal_approx_fast, shuffled across quadrants, one fused DVE
    multiply into bf16 head-padded O^T.
  - Projection: per-head bf16 matmuls against host-zero-padded Wproj over
    the full 128 partitions (O^T pad rows zeroed once on GpSimd at t~0).
"""

import sys

sys.path.insert(0, "/opt/trn_rl_repo")

import numpy as np
import ml_dtypes

B, N, DIM, HEADS = 8, 1024, 1152, 16
HD = DIM // HEADS  # 72
NCORES = 8
QKDIM = 2 * DIM  # 2304 (q and k outdims concatenated)
N_MT_QK = QKDIM // 128  # 18 m-tiles for Q,K
N_KT = DIM // 128  # 9 contraction tiles
N_TT = N // 128  # 8 token tiles
QB = 512  # q block (moving dim) for S^T / qkv
N_QB = N // QB  # 2
VB = 288  # v block = 4 heads
N_VB = DIM // VB  # 4
EB = 384  # proj output block
N_EB = DIM // EB  # 3

_CACHE = {}


def _head_pieces(h):
    """Pieces covering rows [72h, 72h+72) of a 128-row-tiled stack, as
    (mtile, src_lo, src_hi, dst_lo): dest rows [dst_lo, dst_lo+src_hi-src_lo)
    come from src rows [src_lo, src_hi) of mtile."""
    r0 = HD * h
    mt, p0 = divmod(r0, 128)
    ln = min(HD, 128 - p0)
    pieces = [(mt, p0, p0 + ln, 0)]
    if ln < HD:
        pieces.append((mt + 1, 0, HD - ln, ln))
    return pieces


def _build(debug_taps=False):
    import concourse.tile as tile
    from concourse import bacc, mybir

    f32 = mybir.dt.float32
    bf16 = mybir.dt.bfloat16
    Exp = mybir.ActivationFunctionType.Exp

    nc = bacc.Bacc("TRN2", target_bir_lowering=False, debug=False,
                   num_devices=NCORES)

    x_d = nc.dram_tensor("x", [128, N_KT, N], bf16,
                         kind="ExternalInput").ap()  # x^T, host-relayouted
    wqk_d = nc.dram_tensor("wqk", [N_MT_QK, 128, N_KT, 128], bf16,
                           kind="ExternalInput").ap()
    wv_d = nc.dram_tensor("wv", [DIM, DIM], bf16, kind="ExternalInput").ap()
    bqk_d = nc.dram_tensor("bqk", [128, N_MT_QK], f32,
                           kind="ExternalInput").ap()
    bv_d = nc.dram_tensor("bv", [1, DIM], f32, kind="ExternalInput").ap()
    wproj_d = nc.dram_tensor("wproj", [128, N_KT, DIM], bf16,
                             kind="ExternalInput").ap()
    bproj_d = nc.dram_tensor("bproj", [1, DIM], f32,
                             kind="ExternalInput").ap()
    out_d = nc.dram_tensor("out", [N, DIM], f32, kind="ExternalOutput").ap()
    if debug_taps:
        qkt_d = nc.dram_tensor("dbg_qkt", [128, N_MT_QK, N], f32,
                               kind="ExternalOutput").ap()
        vpad_d = nc.dram_tensor("dbg_vpad", [128, N_TT, HEADS, HD + 1], f32,
                                kind="ExternalOutput").ap()
        ot_d = nc.dram_tensor("dbg_ot", [128, N_KT, N], f32,
                              kind="ExternalOutput").ap()
        xt_dbg_d = nc.dram_tensor("dbg_xt", [128, N_KT, N], f32,
                                  kind="ExternalOutput").ap()

    scale = float(HD) ** -0.5

    with tile.TileContext(nc) as tc:
        with tc.tile_pool(name="consts", bufs=1) as consts, \
             tc.tile_pool(name="persist", bufs=1) as persist:
            bqk_sb = consts.tile([128, N_MT_QK], f32)
            nc.sync.dma_start(bqk_sb, bqk_d)
            # biases come in as single rows and are partition-broadcast on
            # the (idle) gpsimd engine -- saves >1MB of startup DMA
            bv_sb = consts.tile([128, DIM], f32)
            nc.sync.dma_start(bv_sb[0:1, :], bv_d)
            nc.gpsimd.partition_broadcast(bv_sb, bv_sb[0:1, :], channels=128)
            bproj_sb = consts.tile([128, DIM], f32)
            nc.sync.dma_start(bproj_sb[0:1, :], bproj_d)
            nc.gpsimd.partition_broadcast(bproj_sb, bproj_sb[0:1, :],
                                          channels=128)

            # Persistent activations
            qkt = persist.tile([128, N_MT_QK, N], bf16)      # Q^T,K^T stacked
            vpad = persist.tile([128, N_TT, HEADS, HD + 1], bf16)
            nc.vector.memset(vpad[:, :, :, HD:HD + 1], 1.0)  # denom trick

            # O^T in COMPACT stacked layout [dim-row, token] (like qkt):
            # written by partition-shifting SBUF->SBUF DMAs from the
            # per-head normalize output, so the projection contracts over
            # 9 full 128-row chunks instead of 16 head-padded ones.
            ot_stack = tc.tile_pool(name="ot_pool", bufs=1)
            ot_pool = ot_stack.__enter__()
            ot = ot_pool.tile([128, N_KT, N], bf16)

            # Repack staging opened at t0 so the head-0 repack DMAs can run
            # during the phase-1 tail.
            pad_stack = tc.tile_pool(name="qk_pad", bufs=1)
            qk_pad_pool = pad_stack.__enter__()

            # ---------------- Phase 1: x^T + QKV projections ---------------
            with tc.tile_pool(name="xt_pool", bufs=1) as xt_pool, \
                 tc.tile_pool(name="wv_pool", bufs=1) as wv_pool, \
                 tc.tile_pool(name="wqk_pool", bufs=4) as wqk_pool, \
                 tc.tile_pool(name="ph1_ps", bufs=1, space="PSUM") as ph1_ps:
                xt = xt_pool.tile([128, N_KT, N], bf16)  # x^T [dim, tok]
                # split by q-block and kt-thirds: parallel queues land the
                # first QK matmul's inputs sooner
                for qb in range(N_QB):
                    for k3 in range(3):
                        nc.sync.dma_start(
                            xt[:, 3 * k3:3 * k3 + 3, qb * QB:(qb + 1) * QB],
                            x_d[:, 3 * k3:3 * k3 + 3, qb * QB:(qb + 1) * QB])
                wv_sb = wv_pool.tile([128, N_KT, DIM], bf16)

                # Q^T / K^T: m-tiles of 128 outdims
                for m in range(N_MT_QK):
                    if m == 4:
                        # wv load deferred behind the first wqk prefetches
                        nc.sync.dma_start(
                            wv_sb,
                            wv_d.rearrange("(kt kp) v -> kp kt v", kp=128))
                    w_t = wqk_pool.tile([128, N_KT, 128], bf16, tag="w")
                    if m < 2:
                        # kt-chunked so the first accumulation matmul's
                        # weights land as early as possible
                        for k3 in range(3):
                            nc.sync.dma_start(
                                w_t[:, 3 * k3:3 * k3 + 3, :],
                                wqk_d[m, :, 3 * k3:3 * k3 + 3, :])
                    else:
                        nc.sync.dma_start(w_t, wqk_d[m])
                    for qb in range(N_QB):
                        ps = ph1_ps.tile([128, QB], f32, tag="qk", bufs=4)
                        for kt in range(N_KT):
                            nc.tensor.matmul(
                                ps,
                                lhsT=w_t[:, kt, :],
                                rhs=xt[:, kt, qb * QB:(qb + 1) * QB],
                                start=(kt == 0), stop=(kt == N_KT - 1))
                        nc.scalar.add(
                            qkt[:, m, qb * QB:(qb + 1) * QB], ps,
                            bqk_sb[:, m:m + 1])

                if debug_taps:
                    nc.gpsimd.dma_start(xt_dbg_d, xt)

                # V in natural layout, 4 heads (288 dims) per block;
                # vb outer so early heads' V completes first and AV can
                # overlap the V-phase tail
                for vb in range(N_VB):
                    for tt in range(N_TT):
                        ps = ph1_ps.tile([128, VB], f32, tag="v", bufs=4)
                        for kt in range(N_KT):
                            nc.tensor.matmul(
                                ps,
                                lhsT=xt[:, kt, tt * 128:(tt + 1) * 128],
                                rhs=wv_sb[:, kt, vb * VB:(vb + 1) * VB],
                                start=(kt == 0), stop=(kt == N_KT - 1))
                        nc.vector.tensor_add(
                            vpad[:, tt, 4 * vb:4 * vb + 4, 0:HD],
                            ps.rearrange("p (g d) -> p g d", g=4),
                            bv_sb[:, vb * VB:(vb + 1) * VB].rearrange(
                                "p (g d) -> p g d", g=4))

            if debug_taps:
                with tc.tile_pool(name="dbg_pool", bufs=2) as dbg_pool:
                    nc.gpsimd.dma_start(qkt_d, qkt)
                    for tt in range(N_TT):
                        cv = dbg_pool.tile([128, HEADS, HD + 1], f32, tag="cv")
                        nc.vector.tensor_copy(cv, vpad[:, tt])
                        nc.sync.dma_start(vpad_d[:, tt], cv)

            # Wproj is prefetched mid-attention (its space frees after
            # phase 1; issuing the DMAs at the boundary floods the queues
            # that the repack DMAs need).
            wp_stack = tc.tile_pool(name="wp_pool", bufs=1)
            wp_pool = wp_stack.__enter__()
            wp_sb = wp_pool.tile([128, N_KT, DIM], bf16)

            # ---------------- Phase 2: attention --------------------------
            # qb outer: after qb==0 all of O^T[:, :, 0:512] is final, so the
            # scheduler can interleave proj matmuls for token tiles 0..3
            # into the ACT-bound qb==1 window (proj pools are open below).
            with tc.tile_pool(name="es_pool", bufs=N_TT + 4) as es_pool, \
                 tc.tile_pool(name="r_pool", bufs=5) as r_pool, \
                 tc.tile_pool(name="out_pool", bufs=3) as out_pool, \
                 tc.tile_pool(name="s_ps", bufs=2, space="PSUM") as s_ps, \
                 tc.tile_pool(name="o_ps", bufs=2, space="PSUM") as o_ps, \
                 tc.tile_pool(name="p_ps", bufs=2, space="PSUM") as p_ps:
                for qb in range(N_QB):
                    for h in range(HEADS):
                        pieces = _head_pieces(h)
                        # repack K^T rows of head h to partitions [0, 72)
                        # with SBUF->SBUF DMA (shifts partitions freely; DMA
                        # engines are idle during attention)
                        ktp = qk_pad_pool.tile([128, N], bf16, tag="ktp",
                                               bufs=4)
                        for (mt, lo, hi, dst) in pieces:
                            nc.sync.dma_start(
                                ktp[dst:dst + hi - lo, :],
                                qkt[lo:hi, 9 + mt, :])
                        if qb == 0 and h == 8:
                            # prefetch Wproj on the gpsimd queue mid-attention
                            for dt in range(N_KT):
                                nc.gpsimd.dma_start(
                                    wp_sb[:, dt, :], wproj_d[:, dt, :])
                        # repack Q^T rows of head h to partitions [0, 72)
                        qtp = qk_pad_pool.tile([128, QB], bf16, tag="qtp",
                                               bufs=6)
                        for (mt, lo, hi, dst) in pieces:
                            nc.gpsimd.dma_start(
                                qtp[dst:dst + hi - lo, :],
                                qkt[lo:hi, mt, qb * QB:(qb + 1) * QB])

                        es_tiles = []
                        for kp in range(N_TT // 2):
                            ps = s_ps.tile([128, 2, QB], f32, tag="s")
                            for j in range(2):
                                kt = 2 * kp + j
                                nc.tensor.matmul(
                                    ps[:, j, :],
                                    lhsT=ktp[0:HD, kt * 128:(kt + 1) * 128],
                                    rhs=qtp[0:HD, :],
                                    start=True, stop=True)
                            es = es_pool.tile([128, 2, QB], bf16, tag="e")
                            nc.scalar.activation(es, ps, func=Exp, scale=scale)
                            es_tiles.append(es)
                        ops = o_ps.tile([128, QB], f32, tag="o")
                        for kt in range(N_TT):
                            nc.tensor.matmul(
                                ops[0:HD + 1, :],
                                lhsT=vpad[:, kt, h, :],
                                rhs=es_tiles[kt // 2][:, kt % 2, :],
                                start=(kt == 0), stop=(kt == N_TT - 1))
                        # Broadcast 1/denominator (psum row 72) to rows 0..72
                        # at 32-aligned bases only: copy the quadrant to SBUF,
                        # shuffle lane 8 down to quadrant 0, invert there
                        # (custom DVE ops need base partition 0), then shuffle
                        # the reciprocal across quadrants.
                        rt = r_pool.tile([96, 2 * QB], f32, tag="r")
                        nc.vector.stream_shuffle(
                            rt[0:32, 0:QB], ops[64:96, :], mask=[8] * 32)
                        nc.vector.reciprocal_approx_fast(
                            rt[0:32, QB:2 * QB], rt[0:32, 0:QB])
                        ident = list(range(32))
                        nc.vector.stream_shuffle(
                            rt[32:64, QB:2 * QB], rt[0:32, QB:2 * QB],
                            mask=ident)
                        nc.vector.stream_shuffle(
                            rt[64:96, QB:2 * QB], rt[0:32, QB:2 * QB],
                            mask=ident)
                        otp = qk_pad_pool.tile([128, QB], bf16, tag="otp",
                                               bufs=4)
                        nc.vector.tensor_mul(
                            otp[0:HD, :], ops[0:HD, :], rt[0:HD, QB:2 * QB])
                        cdma = nc.gpsimd if qb == 0 else nc.sync
                        for (mt, lo, hi, dst) in pieces:
                            cdma.dma_start(
                                ot[lo:hi, mt, qb * QB:(qb + 1) * QB],
                                otp[dst:dst + hi - lo, :])

                # ------------- Phase 3: output projection -------------
                # Inside the same pool scope so proj matmuls for token tiles
                # 0..3 (ready after qb==0) can interleave with qb==1.
                for tt in range(N_TT):
                    outs = out_pool.tile([128, DIM], f32, tag="out")
                    for eb in range(N_EB):
                        ps = p_ps.tile([128, EB], f32, tag="p")
                        for dt in range(N_KT):
                            nc.tensor.matmul(
                                ps,
                                lhsT=ot[:, dt, tt * 128:(tt + 1) * 128],
                                rhs=wp_sb[:, dt, eb * EB:(eb + 1) * EB],
                                start=(dt == 0), stop=(dt == N_KT - 1))
                        nc.vector.tensor_add(
                            outs[:, eb * EB:(eb + 1) * EB], ps,
                            bproj_sb[:, eb * EB:(eb + 1) * EB])
                        nc.sync.dma_start(
                            out_d[tt * 128:(tt + 1) * 128,
                                  eb * EB:(eb + 1) * EB],
                            outs[:, eb * EB:(eb + 1) * EB])

            if debug_taps:
                with tc.tile_pool(name="dbg_pool2", bufs=2) as dbg_pool2:
                    for hh in range(N_KT):
                        co = dbg_pool2.tile([128, N], f32, tag="co")
                        nc.vector.tensor_copy(co, ot[:, hh])
                        nc.sync.dma_start(ot_d[:, hh], co)

            wp_stack.__exit__(None, None, None)
            pad_stack.__exit__(None, None, None)
            ot_stack.__exit__(None, None, None)

    nc.compile()
    return nc


def _get_nc(debug_taps=False):
    key = ("nc", debug_taps)
    if key not in _CACHE:
        _CACHE[key] = _build(debug_taps)
    return _CACHE[key]


def _prep_shared(Wqkv, bqkv, Wproj, bproj):
    """Host-side pure-layout transforms of the (replicated) weights."""
    Wqkv = np.asarray(Wqkv, dtype=np.float32)
    bqkv = np.asarray(bqkv, dtype=np.float32)
    Wproj = np.asarray(Wproj, dtype=np.float32)
    bproj = np.asarray(bproj, dtype=np.float32)

    # [m, kp, kt, o]: per-m-tile, per-partition contiguous
    wqk = np.ascontiguousarray(
        Wqkv[:, :QKDIM].reshape(N_KT, 128, N_MT_QK, 128).transpose(2, 1, 0, 3)
    ).astype(ml_dtypes.bfloat16)
    wv = np.ascontiguousarray(Wqkv[:, QKDIM:]).astype(ml_dtypes.bfloat16)
    bqk = np.ascontiguousarray(bqkv[:QKDIM].reshape(N_MT_QK, 128).T)
    bv = np.ascontiguousarray(bqkv[QKDIM:].reshape(1, DIM))
    # compact Wproj: wproj[kp, kt, e] = Wproj[kt*128+kp, e]
    wproj = np.ascontiguousarray(
        Wproj.reshape(N_KT, 128, DIM).transpose(1, 0, 2)).astype(
            ml_dtypes.bfloat16)
    bproj2 = np.ascontiguousarray(bproj.reshape(1, DIM))
    return dict(wqk=wqk, wv=wv, bqk=bqk, bv=bv, wproj=wproj, bproj=bproj2)


def kernel(x, Wqkv, bqkv, Wproj, bproj, _trace=False, _debug_taps=False):
    from concourse import bass_utils

    x = np.asarray(x, dtype=np.float32).astype(ml_dtypes.bfloat16)
    # device layout: xt[p, dt, t] = x[t, dt*128+p]
    xt = x.transpose(0, 2, 1).reshape(NCORES, N_KT, 128, N).transpose(
        0, 2, 1, 3)
    shared = _prep_shared(Wqkv, bqkv, Wproj, bproj)
    in_maps = [dict(x=np.ascontiguousarray(xt[i]), **shared)
               for i in range(NCORES)]
    nc = _get_nc(_debug_taps)
    res = bass_utils.run_bass_kernel_spmd(
        nc, in_maps, core_ids=list(range(NCORES)), trace=_trace)
    out = np.stack([res.results[i]["out"] for i in range(NCORES)], axis=0)
    if _trace:
        _CACHE["last_exec_time_ns"] = res.exec_time_ns
        _CACHE["last_results"] = res
    return out



# revision 2
# speedup vs baseline: 1.0104x; 1.0104x over previous
"""Multi-head attention (B=8, N=1024, DIM=1152, H=16, hd=72) on 8 TRN2 cores.

Sharding: pure data parallelism -- core i computes batch element i, weights
replicated, no collectives.

v2 over the phase-structured baseline: one globally software-pipelined
emission schedule built by a greedy virtual-clock emitter.

  - Startup: x and the first QK weight tiles stream on separate DMA queues
    (sync / scalar) with the first-needed chunks issued first, so the first
    matmul starts at ~10us instead of ~21us.
  - QK m-tiles are emitted in K/Q pairs (m9+j, mj) on demand; attention
    units (per head, per 512-token q-block) start as soon as their two
    stacked-QK m-tiles exist -- S/exp/AV of early heads overlap the rest of
    the QKV projection.
  - exp on ScalarE is the per-unit bottleneck (4.5us vs 3.44us of S+AV
    tensor work), so the emitter paces S-units by a virtual scalar clock
    and fills the gaps with QK pairs / V blocks / proj tiles; leftover
    fillers are spent where attention alone would stall on exp.
  - QKV psum evictions move to DVE (tensor_scalar_add with the per-partition
    bias column) leaving ScalarE exp-only.
  - proj(tt 0..3) runs as filler during qb1 attention; only proj(tt 4..7)
    trails the last attention unit.

Numerics identical to baseline: bf16 matmuls, fp32 psum, exp without max
subtraction (scores ~N(0,1)), softmax denominator via ones-column in the
AV matmul, reciprocal broadcast via stream_shuffle quadrants.
"""

import sys

sys.path.insert(0, "/opt/trn_rl_repo")

import numpy as np
import ml_dtypes

B, N, DIM, HEADS = 8, 1024, 1152, 16
HD = DIM // HEADS  # 72
NCORES = 8
QKDIM = 2 * DIM  # 2304
N_MT_QK = QKDIM // 128  # 18
N_KT = DIM // 128  # 9
N_TT = N // 128  # 8
QB = 512
N_QB = N // QB  # 2
VB = 288  # v block = 4 heads
N_VB = DIM // VB  # 4
EB = 384
N_EB = DIM // EB  # 3

_CACHE = {}

# virtual-clock cost estimates (ns)
MM512 = 215
MM288 = 121
MM384 = 163
EXP_T = 1125
S_UNIT = 8 * MM512
AV_UNIT = 8 * MM512


def _head_pieces(h):
    """Pieces covering rows [72h, 72h+72) of a 128-row-tiled stack."""
    r0 = HD * h
    mt, p0 = divmod(r0, 128)
    ln = min(HD, 128 - p0)
    pieces = [(mt, p0, p0 + ln, 0)]
    if ln < HD:
        pieces.append((mt + 1, 0, HD - ln, ln))
    return pieces


def _head_tiles(h):
    return sorted({mt for (mt, _, _, _) in _head_pieces(h)})


def _build(verbose=False):
    import concourse.tile as tile
    from concourse import bacc, mybir

    f32 = mybir.dt.float32
    bf16 = mybir.dt.bfloat16
    Exp = mybir.ActivationFunctionType.Exp

    nc = bacc.Bacc("TRN2", target_bir_lowering=False, debug=False,
                   num_devices=NCORES)

    x_d = nc.dram_tensor("x", [128, N_KT, N], bf16,
                         kind="ExternalInput").ap()
    wqk_d = nc.dram_tensor("wqk", [N_MT_QK, 128, N_KT, 128], bf16,
                           kind="ExternalInput").ap()
    wv_d = nc.dram_tensor("wv", [DIM, DIM], bf16, kind="ExternalInput").ap()
    bqk_d = nc.dram_tensor("bqk", [128, N_MT_QK], f32,
                           kind="ExternalInput").ap()
    bv_d = nc.dram_tensor("bv", [1, DIM], f32, kind="ExternalInput").ap()
    wproj_d = nc.dram_tensor("wproj", [128, N_KT, DIM], bf16,
                             kind="ExternalInput").ap()
    bproj_d = nc.dram_tensor("bproj", [1, DIM], f32,
                             kind="ExternalInput").ap()
    out_d = nc.dram_tensor("out", [N, DIM], f32, kind="ExternalOutput").ap()

    scale = float(HD) ** -0.5

    with tile.TileContext(nc) as tc:
        with tc.tile_pool(name="consts", bufs=1) as consts, \
             tc.tile_pool(name="persist", bufs=1) as persist, \
             tc.tile_pool(name="xt_pool", bufs=1) as xt_pool, \
             tc.tile_pool(name="wv_pool", bufs=1) as wv_pool, \
             tc.tile_pool(name="wqk_pool", bufs=8) as wqk_pool, \
             tc.tile_pool(name="wp_pool", bufs=1) as wp_pool, \
             tc.tile_pool(name="pad", bufs=1) as pad, \
             tc.tile_pool(name="es_pool", bufs=12) as es_pool, \
             tc.tile_pool(name="r_pool", bufs=2) as r_pool, \
             tc.tile_pool(name="out_pool", bufs=2) as out_pool, \
             tc.tile_pool(name="qp_ps", bufs=2, space="PSUM") as qp_ps, \
             tc.tile_pool(name="s_ps", bufs=2, space="PSUM") as s_ps, \
             tc.tile_pool(name="ov_ps", bufs=2, space="PSUM") as ov_ps:

            # ---------------- prologue DMAs ----------------
            # x on sync: qb0 chunks first (first QK pair needs them)
            xt = xt_pool.tile([128, N_KT, N], bf16)
            for qb in range(N_QB):
                nc.sync.dma_start(xt[:, 0:3, qb * QB:(qb + 1) * QB],
                                  x_d[:, 0:3, qb * QB:(qb + 1) * QB])
                nc.sync.dma_start(xt[:, 3:9, qb * QB:(qb + 1) * QB],
                                  x_d[:, 3:9, qb * QB:(qb + 1) * QB])
            bqk_sb = consts.tile([128, N_MT_QK], f32)
            nc.sync.dma_start(bqk_sb, bqk_d)

            # first QK weight tiles on the scalar queue (idle until exp)
            w_tiles = {}

            def issue_w(m, chunked, eng):
                w_t = wqk_pool.tile([128, N_KT, 128], bf16, tag="w")
                if chunked:
                    eng.dma_start(w_t[:, 0:3, :], wqk_d[m, :, 0:3, :])
                    eng.dma_start(w_t[:, 3:9, :], wqk_d[m, :, 3:9, :])
                else:
                    eng.dma_start(w_t, wqk_d[m])
                w_tiles[m] = w_t

            issue_w(9, True, nc.scalar)
            issue_w(0, True, nc.scalar)
            issue_w(10, False, nc.scalar)
            issue_w(1, False, nc.scalar)
            issue_w(11, False, nc.scalar)
            issue_w(2, False, nc.scalar)

            # biases + wv on gpsimd
            bv_sb = consts.tile([128, DIM], f32)
            nc.gpsimd.dma_start(bv_sb[0:1, :], bv_d)
            nc.gpsimd.partition_broadcast(bv_sb, bv_sb[0:1, :], channels=128)
            bproj_sb = consts.tile([128, DIM], f32)
            nc.gpsimd.dma_start(bproj_sb[0:1, :], bproj_d)
            nc.gpsimd.partition_broadcast(bproj_sb, bproj_sb[0:1, :],
                                          channels=128)
            # wv split by v-block; every block marker-gated off a qkt
            # eviction so none of it crowds the x/wqk startup window
            wv_sb = wv_pool.tile([128, N_KT, DIM], bf16)
            wv_r = wv_d.rearrange("(kt kp) v -> kp kt v", kp=128)

            # persistent activations
            qkt = persist.tile([128, N_MT_QK, N], bf16)
            vpad = persist.tile([128, N_TT, HEADS, HD + 1], bf16)
            nc.vector.memset(vpad[:, :, :, HD:HD + 1], 1.0)
            ot = persist.tile([128, N_KT, N], bf16)
            wp_sb = wp_pool.tile([128, N_KT, DIM], bf16)

            # ---------------- virtual-clock emitter ----------------
            T = {"tensor": 10000.0, "scalar": 10000.0}
            exp_done = {}
            es_tiles = {}
            # availability estimates (ns), W_RDY keyed by m-tile
            X_RDY = {0: 14000.0, 1: 20000.0}  # full x for q-block
            W_RDY = {9: 12500.0, 0: 14000.0, 10: 17000.0, 1: 18500.0,
                     11: 20500.0, 2: 22000.0}
            WV_RDY = [30000.0, 60000.0, 80000.0, 100000.0]  # per vb
            REPACK_NS = 5500.0  # evict -> sbuf repack DMA latency
            q_done = {}
            timeline = []

            def log_ev(kind, tag):
                timeline.append((T["tensor"], T["scalar"], kind, tag))

            def emit_qk_mtile(m, qb):
                T["tensor"] = max(T["tensor"], X_RDY[qb])
                ps = qp_ps.tile([128, QB], f32, tag="qp")
                w_t = w_tiles[m]
                for kt in range(N_KT):
                    nc.tensor.matmul(
                        ps, lhsT=w_t[:, kt, :],
                        rhs=xt[:, kt, qb * QB:(qb + 1) * QB],
                        start=(kt == 0), stop=(kt == N_KT - 1))
                nc.vector.tensor_scalar_add(
                    qkt[:, m, qb * QB:(qb + 1) * QB], ps, bqk_sb[:, m:m + 1])
                T["tensor"] += 9 * MM512

            def emit_qk_quarter(m, qb):
                """One (m-tile, q-block) = 9 matmuls + DVE evict (1.94us)."""
                T["tensor"] = max(T["tensor"], X_RDY[qb],
                                  W_RDY.get(m, 0.0))
                emit_qk_mtile(m, qb)
                q_done[(m, qb)] = T["tensor"]
                log_ev("qk", (m, qb))

            def emit_v_tt(vb, tt):
                """One V token-tile of a 4-head block (1.09us)."""
                T["tensor"] = max(T["tensor"], WV_RDY[vb], X_RDY[1])
                ps = ov_ps.tile([128, QB], f32, tag="ov")
                for kt in range(N_KT):
                    nc.tensor.matmul(
                        ps[:, 0:VB],
                        lhsT=xt[:, kt, tt * 128:(tt + 1) * 128],
                        rhs=wv_sb[:, kt, vb * VB:(vb + 1) * VB],
                        start=(kt == 0), stop=(kt == N_KT - 1))
                nc.vector.tensor_add(
                    vpad[:, tt, 4 * vb:4 * vb + 4, 0:HD],
                    ps[:, 0:VB].rearrange("p (g d) -> p g d", g=4),
                    bv_sb[:, vb * VB:(vb + 1) * VB].rearrange(
                        "p (g d) -> p g d", g=4))
                T["tensor"] += 9 * MM288
                log_ev("V", (vb, tt))

            def repack_ready(h, qb):
                """Estimated time the head's repack DMAs can have finished."""
                t = 0.0
                for mt in _head_tiles(h):
                    for q in range(N_QB):
                        t = max(t, q_done.get((9 + mt, q), 0.0))
                    t = max(t, q_done.get((mt, qb), 0.0))
                return t + REPACK_NS

            def emit_unit_s(h, qb):
                T["tensor"] = max(T["tensor"], repack_ready(h, qb))
                pieces = _head_pieces(h)
                ktp = pad.tile([128, N], bf16, tag="ktp", bufs=2)
                for (mt, lo, hi, dst) in pieces:
                    nc.sync.dma_start(ktp[dst:dst + hi - lo, :],
                                      qkt[lo:hi, 9 + mt, :])
                qtp = pad.tile([128, QB], bf16, tag="qtp", bufs=4)
                for (mt, lo, hi, dst) in pieces:
                    nc.gpsimd.dma_start(
                        qtp[dst:dst + hi - lo, :],
                        qkt[lo:hi, mt, qb * QB:(qb + 1) * QB])
                ets = []
                for kp in range(N_TT // 2):
                    ps = s_ps.tile([128, 2, QB], f32, tag="s")
                    for j in range(2):
                        kt = 2 * kp + j
                        nc.tensor.matmul(
                            ps[:, j, :],
                            lhsT=ktp[0:HD, kt * 128:(kt + 1) * 128],
                            rhs=qtp[0:HD, :],
                            start=True, stop=True)
                    es = es_pool.tile([128, 2, QB], bf16, tag="e")
                    nc.scalar.activation(es, ps, func=Exp, scale=scale)
                    ets.append(es)
                    T["tensor"] += 2 * MM512
                    T["scalar"] = max(T["scalar"], T["tensor"]) + EXP_T
                es_tiles[(h, qb)] = ets
                exp_done[(h, qb)] = T["scalar"]
                log_ev("S", (h, qb))

            def emit_unit_av(h, qb):
                pieces = _head_pieces(h)
                ets = es_tiles.pop((h, qb))
                ops = ov_ps.tile([128, QB], f32, tag="ov")
                for kt in range(N_TT):
                    nc.tensor.matmul(
                        ops[0:HD + 1, :],
                        lhsT=vpad[:, kt, h, :],
                        rhs=ets[kt // 2][:, kt % 2, :],
                        start=(kt == 0), stop=(kt == N_TT - 1))
                rt = r_pool.tile([96, 2 * QB], f32, tag="r")
                nc.vector.stream_shuffle(
                    rt[0:32, 0:QB], ops[64:96, :], mask=[8] * 32)
                nc.vector.reciprocal_approx_fast(
                    rt[0:32, QB:2 * QB], rt[0:32, 0:QB])
                ident = list(range(32))
                nc.vector.stream_shuffle(
                    rt[32:64, QB:2 * QB], rt[0:32, QB:2 * QB], mask=ident)
                nc.vector.stream_shuffle(
                    rt[64:96, QB:2 * QB], rt[0:32, QB:2 * QB], mask=ident)
                otp = pad.tile([128, QB], bf16, tag="otp", bufs=4)
                nc.vector.tensor_mul(
                    otp[0:HD, :], ops[0:HD, :], rt[0:HD, QB:2 * QB])
                cdma = nc.sync if qb == 0 else nc.gpsimd
                for (mt, lo, hi, dst) in pieces:
                    cdma.dma_start(
                        ot[lo:hi, mt, qb * QB:(qb + 1) * QB],
                        otp[dst:dst + hi - lo, :])
                T["tensor"] = max(T["tensor"] + 8 * MM512,
                                  exp_done[(h, qb)] + 2 * MM512)
                log_ev("AV", (h, qb))

            out_tiles = {}

            def emit_proj_eb(tt, eb):
                """One proj (token-tile, out-block): 9 matmuls (1.47us)."""
                outs = out_pool.tile([128, EB], f32, tag="out")
                ps = qp_ps.tile([128, QB], f32, tag="qp")
                for dt in range(N_KT):
                    nc.tensor.matmul(
                        ps[:, 0:EB],
                        lhsT=ot[:, dt, tt * 128:(tt + 1) * 128],
                        rhs=wp_sb[:, dt, eb * EB:(eb + 1) * EB],
                        start=(dt == 0), stop=(dt == N_KT - 1))
                nc.vector.tensor_add(
                    outs, ps[:, 0:EB], bproj_sb[:, eb * EB:(eb + 1) * EB])
                nc.sync.dma_start(
                    out_d[tt * 128:(tt + 1) * 128, eb * EB:(eb + 1) * EB],
                    outs)
                T["tensor"] += 9 * MM384
                log_ev("proj", (tt, eb))

            # ---------------- schedule ----------------
            # unit order: all qb0 units (h ascending), then all qb1
            units = [(h, 0) for h in range(HEADS)] + \
                    [(h, 1) for h in range(HEADS)]
            s_idx = 0
            av_idx = 0
            # fine-grained filler state
            qk_left = []            # (m, qb) quarters in preferred order
            for j in range(9):
                for m in (9 + j, j):
                    for qb in range(N_QB):
                        qk_left.append((m, qb))
            qk_emitted = set()
            w_issued = {9, 0, 10, 1, 11, 2}
            v_left = [(vb, tt) for vb in range(N_VB) for tt in range(N_TT)]
            v_done_tt = [0] * N_VB
            # proj quarters: gate 0 = qb0 attended, 1 = qb1 attended,
            # 2 = all AV emitted (reserved tail filler for the last-unit
            # normalize/ot-write latency)
            RESERVE = [(2, 2), (3, 0), (3, 1), (3, 2)]
            proj_list = \
                [(tt, eb, 0) for tt in range(4) for eb in range(N_EB)
                 if (tt, eb) not in RESERVE] + \
                [(tt, eb, 2) for (tt, eb) in RESERVE] + \
                [(tt, eb, 1) for tt in range(4, 8) for eb in range(N_EB)]
            proj_done = set()
            qb_done_av = [0, 0]
            wproj_issued = False
            wv_issued = 0

            def s_needs(u):
                """Unemitted qk quarters needed by S(h, qb)."""
                h, qb = u
                need = []
                for mt in _head_tiles(h):
                    for q in range(N_QB):       # ktp reads both halves
                        if (9 + mt, q) not in qk_emitted:
                            need.append((9 + mt, q))
                    if (mt, qb) not in qk_emitted:
                        need.append((mt, qb))
                return need

            def av_ready(u):
                return v_done_tt[u[0] // 4] == N_TT

            def prefetch_w(m, qb):
                """Keep weight DMAs ~2 m-tiles ahead of use (sync queue)."""
                if qb != 1:
                    return
                base = 9 if m >= 9 else 0
                nm = m + 2
                if base <= nm < base + 9 and nm not in w_issued:
                    issue_w(nm, False, nc.sync)
                    w_issued.add(nm)
                    W_RDY[nm] = T["tensor"] + 6000.0

            def do_qk(m, qb):
                if m not in w_issued:
                    issue_w(m, False, nc.sync)
                    w_issued.add(m)
                    W_RDY[m] = T["tensor"] + 6000.0
                emit_qk_quarter(m, qb)
                qk_emitted.add((m, qb))
                qk_left.remove((m, qb))
                prefetch_w(m, qb)

            def do_v():
                vb, tt = v_left.pop(0)
                emit_v_tt(vb, tt)
                v_done_tt[vb] += 1

            def do_av(u, forced=False):
                if forced:
                    T["tensor"] = max(T["tensor"], exp_done[u])
                emit_unit_av(*u)
                qb_done_av[u[1]] += 1

            while s_idx < 32 or av_idx < 32 or qk_left or v_left \
                    or len(proj_done) < len(proj_list):
                # Deferred weight loads: queue position cannot delay a DMA
                # (the scheduler hoists ready instructions), so gate each
                # load with a marker copy whose SOURCE is a late-written qkt
                # m-tile -- the WAW dep on the marker element holds the DMA
                # until that m-tile's eviction really happened.
                if wv_issued < N_VB and s_idx >= 3 * wv_issued and \
                        (0, 0) in qk_emitted:
                    vb = wv_issued
                    gate_m = {0: 0, 1: 11, 2: 13, 3: 15}[vb]
                    nc.vector.tensor_copy(
                        wv_sb[0:1, 0:1, vb * VB:vb * VB + 1],
                        qkt[0:1, gate_m, 0:1])
                    nc.gpsimd.dma_start(
                        wv_sb[:, :, vb * VB:(vb + 1) * VB],
                        wv_r[:, :, vb * VB:(vb + 1) * VB])
                    wv_issued += 1
                if not wproj_issued and s_idx >= 14:
                    for c in range(3):
                        nc.vector.tensor_copy(
                            wp_sb[0:1, 3 * c:3 * c + 1, 0:1],
                            qkt[0:1, 17, 0:1])
                        nc.gpsimd.dma_start(
                            wp_sb[:, 3 * c:3 * c + 3, :],
                            wproj_d[:, 3 * c:3 * c + 3, :])
                    wproj_issued = True

                # 1. AV whose exp is done and V block complete
                if av_idx < s_idx:
                    u = units[av_idx]
                    if av_ready(u) and exp_done[u] <= T["tensor"] + 300:
                        do_av(u)
                        av_idx += 1
                        continue
                # 2. S paced by scalar backlog and repack latency
                if s_idx < 32 and s_idx - av_idx < 2:
                    u = units[s_idx]
                    if not s_needs(u) and \
                            T["scalar"] - T["tensor"] < 2200 and \
                            repack_ready(*u) <= T["tensor"] + 400:
                        emit_unit_s(*u)
                        s_idx += 1
                        continue
                # 3. fillers, finest-urgency first
                # 3a. qk quarters needed by the next S unit
                if s_idx < 32:
                    need = s_needs(units[s_idx])
                    if need:
                        m, qb = need[0]
                        if W_RDY.get(m, 0.0) <= T["tensor"] + 2500:
                            do_qk(m, qb)
                            continue
                # 3b. V tiles for the next blocked AV
                if av_idx < s_idx and not av_ready(units[av_idx]) and \
                        v_left and T["tensor"] >= WV_RDY[v_left[0][0]] - 4000:
                    do_v()
                    continue
                # 3c. proj blocks whose gate is satisfied
                if len(proj_done) < len(proj_list) and wproj_issued:
                    hit = None
                    for (tt, eb, gate) in proj_list:
                        if (tt, eb) in proj_done:
                            continue
                        if gate == 2:
                            ok = av_idx == 32
                        else:
                            ok = qb_done_av[gate] == HEADS
                        if ok:
                            hit = (tt, eb)
                            break
                    if hit is not None:
                        emit_proj_eb(*hit)
                        proj_done.add(hit)
                        continue
                # 3d. spare qk quarter
                if qk_left:
                    for (m, qb) in qk_left:
                        if W_RDY.get(m, 0.0) <= T["tensor"] + 2500:
                            do_qk(m, qb)
                            break
                    else:
                        m, qb = qk_left[0]
                        T["tensor"] = max(T["tensor"], W_RDY.get(m, 0.0))
                        do_qk(m, qb)
                    continue
                # 3e. spare V tile
                if v_left and T["tensor"] >= WV_RDY[v_left[0][0]] - 4000:
                    do_v()
                    continue
                # 4. forced (stall)
                if av_idx < s_idx and av_ready(units[av_idx]):
                    do_av(units[av_idx], forced=True)
                    av_idx += 1
                    continue
                if s_idx < 32 and not s_needs(units[s_idx]):
                    emit_unit_s(*units[s_idx])
                    s_idx += 1
                    continue
                if v_left:
                    T["tensor"] = max(T["tensor"], WV_RDY[v_left[0][0]])
                    do_v()
                    continue
                # last resort: S with repack stall
                if s_idx < 32 and not s_needs(units[s_idx]):
                    u = units[s_idx]
                    T["tensor"] = max(T["tensor"], repack_ready(*u))
                    emit_unit_s(*u)
                    s_idx += 1
                    continue
                raise RuntimeError("emitter deadlock")

            if verbose:
                for (tt_, ts_, kind, tag) in timeline:
                    print(f"  t={tt_/1000:7.2f}us scalar={ts_/1000:7.2f} "
                          f"{kind:5s} {tag}")
                print(f"  predicted tensor end: {T['tensor']/1000:.1f}us")

    nc.compile()
    return nc


def _get_nc():
    if "nc" not in _CACHE:
        _CACHE["nc"] = _build(verbose=False)
    return _CACHE["nc"]


def _prep_shared(Wqkv, bqkv, Wproj, bproj):
    Wqkv = np.asarray(Wqkv, dtype=np.float32)
    bqkv = np.asarray(bqkv, dtype=np.float32)
    Wproj = np.asarray(Wproj, dtype=np.float32)
    bproj = np.asarray(bproj, dtype=np.float32)

    wqk = np.ascontiguousarray(
        Wqkv[:, :QKDIM].reshape(N_KT, 128, N_MT_QK, 128).transpose(2, 1, 0, 3)
    ).astype(ml_dtypes.bfloat16)
    wv = np.ascontiguousarray(Wqkv[:, QKDIM:]).astype(ml_dtypes.bfloat16)
    bqk = np.ascontiguousarray(bqkv[:QKDIM].reshape(N_MT_QK, 128).T)
    bv = np.ascontiguousarray(bqkv[QKDIM:].reshape(1, DIM))
    wproj = np.ascontiguousarray(
        Wproj.reshape(N_KT, 128, DIM).transpose(1, 0, 2)).astype(
            ml_dtypes.bfloat16)
    bproj2 = np.ascontiguousarray(bproj.reshape(1, DIM))
    return dict(wqk=wqk, wv=wv, bqk=bqk, bv=bv, wproj=wproj, bproj=bproj2)


def kernel(x, Wqkv, bqkv, Wproj, bproj, _trace=False):
    from concourse import bass_utils

    x = np.asarray(x, dtype=np.float32).astype(ml_dtypes.bfloat16)
    xt = x.transpose(0, 2, 1).reshape(NCORES, N_KT, 128, N).transpose(
        0, 2, 1, 3)
    shared = _prep_shared(Wqkv, bqkv, Wproj, bproj)
    in_maps = [dict(x=np.ascontiguousarray(xt[i]), **shared)
               for i in range(NCORES)]
    nc = _get_nc()
    res = bass_utils.run_bass_kernel_spmd(
        nc, in_maps, core_ids=list(range(NCORES)), trace=_trace)
    out = np.stack([res.results[i]["out"] for i in range(NCORES)], axis=0)
    if _trace:
        _CACHE["last_exec_time_ns"] = res.exec_time_ns
        _CACHE["last_results"] = res
    return out


if __name__ == "__main__":
    _build(verbose=True)


# revision 3
# speedup vs baseline: 1.0131x; 1.0027x over previous
"""Multi-head attention (B=8, N=1024, DIM=1152, H=16, hd=72) on 8 TRN2 cores.

Sharding: pure data parallelism -- core i computes batch element i, weights
replicated, no collectives.

v2 over the phase-structured baseline: one globally software-pipelined
emission schedule built by a greedy virtual-clock emitter.

  - Startup: x and the first QK weight tiles stream on separate DMA queues
    (sync / scalar) with the first-needed chunks issued first, so the first
    matmul starts at ~10us instead of ~21us.
  - QK m-tiles are emitted in K/Q pairs (m9+j, mj) on demand; attention
    units (per head, per 512-token q-block) start as soon as their two
    stacked-QK m-tiles exist -- S/exp/AV of early heads overlap the rest of
    the QKV projection.
  - exp on ScalarE is the per-unit bottleneck (4.5us vs 3.44us of S+AV
    tensor work), so the emitter paces S-units by a virtual scalar clock
    and fills the gaps with QK pairs / V blocks / proj tiles; leftover
    fillers are spent where attention alone would stall on exp.
  - QKV psum evictions move to DVE (tensor_scalar_add with the per-partition
    bias column) leaving ScalarE exp-only.
  - proj(tt 0..3) runs as filler during qb1 attention; only proj(tt 4..7)
    trails the last attention unit.

Numerics identical to baseline: bf16 matmuls, fp32 psum, exp without max
subtraction (scores ~N(0,1)), softmax denominator via ones-column in the
AV matmul, reciprocal broadcast via stream_shuffle quadrants.
"""

import sys

sys.path.insert(0, "/opt/trn_rl_repo")

import numpy as np
import ml_dtypes

B, N, DIM, HEADS = 8, 1024, 1152, 16
HD = DIM // HEADS  # 72
NCORES = 8
QKDIM = 2 * DIM  # 2304
N_MT_QK = QKDIM // 128  # 18
N_KT = DIM // 128  # 9
N_TT = N // 128  # 8
QB = 512
N_QB = N // QB  # 2
VB = 288  # v block = 4 heads
N_VB = DIM // VB  # 4
EB = 384
N_EB = DIM // EB  # 3

_CACHE = {}

# virtual-clock cost estimates (ns)
MM512 = 215
MM288 = 121
MM384 = 163
EXP_T = 1125
S_UNIT = 8 * MM512
AV_UNIT = 8 * MM512


def _head_pieces(h):
    """Pieces covering rows [72h, 72h+72) of a 128-row-tiled stack."""
    r0 = HD * h
    mt, p0 = divmod(r0, 128)
    ln = min(HD, 128 - p0)
    pieces = [(mt, p0, p0 + ln, 0)]
    if ln < HD:
        pieces.append((mt + 1, 0, HD - ln, ln))
    return pieces


def _head_tiles(h):
    return sorted({mt for (mt, _, _, _) in _head_pieces(h)})


def _build(verbose=False):
    import concourse.tile as tile
    from concourse import bacc, mybir

    f32 = mybir.dt.float32
    bf16 = mybir.dt.bfloat16
    Exp = mybir.ActivationFunctionType.Exp

    nc = bacc.Bacc("TRN2", target_bir_lowering=False, debug=False,
                   num_devices=NCORES)

    x_d = nc.dram_tensor("x", [128, N_KT, N], bf16,
                         kind="ExternalInput").ap()
    wqk_d = nc.dram_tensor("wqk", [N_MT_QK, 128, N_KT, 128], bf16,
                           kind="ExternalInput").ap()
    wv_d = nc.dram_tensor("wv", [DIM, DIM], bf16, kind="ExternalInput").ap()
    bqk_d = nc.dram_tensor("bqk", [128, N_MT_QK], f32,
                           kind="ExternalInput").ap()
    bv_d = nc.dram_tensor("bv", [1, DIM], f32, kind="ExternalInput").ap()
    wproj_d = nc.dram_tensor("wproj", [128, N_KT, DIM], bf16,
                             kind="ExternalInput").ap()
    bproj_d = nc.dram_tensor("bproj", [1, DIM], f32,
                             kind="ExternalInput").ap()
    out_d = nc.dram_tensor("out", [N, DIM], f32, kind="ExternalOutput").ap()

    scale = float(HD) ** -0.5

    with tile.TileContext(nc) as tc:
        with tc.tile_pool(name="consts", bufs=1) as consts, \
             tc.tile_pool(name="persist", bufs=1) as persist, \
             tc.tile_pool(name="xt_pool", bufs=1) as xt_pool, \
             tc.tile_pool(name="wv_pool", bufs=1) as wv_pool, \
             tc.tile_pool(name="wqk_pool", bufs=8) as wqk_pool, \
             tc.tile_pool(name="wp_pool", bufs=1) as wp_pool, \
             tc.tile_pool(name="pad", bufs=1) as pad, \
             tc.tile_pool(name="es_pool", bufs=12) as es_pool, \
             tc.tile_pool(name="r_pool", bufs=2) as r_pool, \
             tc.tile_pool(name="out_pool", bufs=2) as out_pool, \
             tc.tile_pool(name="qp_ps", bufs=2, space="PSUM") as qp_ps, \
             tc.tile_pool(name="s_ps", bufs=2, space="PSUM") as s_ps, \
             tc.tile_pool(name="ov_ps", bufs=2, space="PSUM") as ov_ps:

            # ---------------- prologue DMAs ----------------
            # x on sync: qb0 chunks first (first QK pair needs them)
            xt = xt_pool.tile([128, N_KT, N], bf16)
            for qb in range(N_QB):
                nc.sync.dma_start(xt[:, 0:3, qb * QB:(qb + 1) * QB],
                                  x_d[:, 0:3, qb * QB:(qb + 1) * QB])
                nc.sync.dma_start(xt[:, 3:9, qb * QB:(qb + 1) * QB],
                                  x_d[:, 3:9, qb * QB:(qb + 1) * QB])
            bqk_sb = consts.tile([128, N_MT_QK], f32)
            nc.sync.dma_start(bqk_sb, bqk_d)

            # first QK weight tiles on the scalar queue (idle until exp)
            w_tiles = {}

            def issue_w(m, chunked, eng):
                w_t = wqk_pool.tile([128, N_KT, 128], bf16, tag="w")
                if chunked:
                    eng.dma_start(w_t[:, 0:3, :], wqk_d[m, :, 0:3, :])
                    eng.dma_start(w_t[:, 3:9, :], wqk_d[m, :, 3:9, :])
                else:
                    eng.dma_start(w_t, wqk_d[m])
                w_tiles[m] = w_t

            issue_w(9, False, nc.scalar)
            issue_w(0, False, nc.scalar)
            issue_w(10, False, nc.scalar)
            issue_w(1, False, nc.scalar)
            issue_w(11, False, nc.scalar)
            issue_w(2, False, nc.scalar)

            # biases + wv on gpsimd
            bv_sb = consts.tile([128, DIM], f32)
            nc.gpsimd.dma_start(bv_sb[0:1, :], bv_d)
            nc.gpsimd.partition_broadcast(bv_sb, bv_sb[0:1, :], channels=128)
            bproj_sb = consts.tile([128, DIM], f32)
            nc.gpsimd.dma_start(bproj_sb[0:1, :], bproj_d)
            nc.gpsimd.partition_broadcast(bproj_sb, bproj_sb[0:1, :],
                                          channels=128)
            # wv split by v-block; every block marker-gated off a qkt
            # eviction so none of it crowds the x/wqk startup window
            wv_sb = wv_pool.tile([128, N_KT, DIM], bf16)
            wv_r = wv_d.rearrange("(kt kp) v -> kp kt v", kp=128)

            # persistent activations
            qkt = persist.tile([128, N_MT_QK, N], bf16)
            vpad = persist.tile([128, N_TT, HEADS, HD + 1], bf16)
            nc.vector.memset(vpad[:, :, :, HD:HD + 1], 1.0)
            ot = persist.tile([128, N_KT, N], bf16)
            wp_sb = wp_pool.tile([128, N_KT, DIM], bf16)

            # ---------------- virtual-clock emitter ----------------
            T = {"tensor": 10000.0, "scalar": 10000.0}
            exp_done = {}
            es_tiles = {}
            # availability estimates (ns), W_RDY keyed by m-tile
            X_RDY = {0: 14000.0, 1: 20000.0}  # full x for q-block
            W_RDY = {9: 12500.0, 0: 14000.0, 10: 17000.0, 1: 18500.0,
                     11: 20500.0, 2: 22000.0}
            WV_RDY = [30000.0, 60000.0, 80000.0, 100000.0]  # per vb
            REPACK_NS = 5500.0  # evict -> sbuf repack DMA latency
            q_done = {}
            timeline = []

            def log_ev(kind, tag):
                timeline.append((T["tensor"], T["scalar"], kind, tag))

            def emit_qk_mtile(m, qb):
                T["tensor"] = max(T["tensor"], X_RDY[qb])
                ps = qp_ps.tile([128, QB], f32, tag="qp")
                w_t = w_tiles[m]
                for kt in range(N_KT):
                    nc.tensor.matmul(
                        ps, lhsT=w_t[:, kt, :],
                        rhs=xt[:, kt, qb * QB:(qb + 1) * QB],
                        start=(kt == 0), stop=(kt == N_KT - 1))
                nc.vector.tensor_scalar_add(
                    qkt[:, m, qb * QB:(qb + 1) * QB], ps, bqk_sb[:, m:m + 1])
                T["tensor"] += 9 * MM512

            def emit_qk_quarter(m, qb):
                """One (m-tile, q-block) = 9 matmuls + DVE evict (1.94us)."""
                T["tensor"] = max(T["tensor"], X_RDY[qb],
                                  W_RDY.get(m, 0.0))
                emit_qk_mtile(m, qb)
                q_done[(m, qb)] = T["tensor"]
                log_ev("qk", (m, qb))

            def emit_v_tt(vb, tt):
                """One V token-tile of a 4-head block (1.09us)."""
                T["tensor"] = max(T["tensor"], WV_RDY[vb], X_RDY[1])
                ps = ov_ps.tile([128, QB], f32, tag="ov")
                for kt in range(N_KT):
                    nc.tensor.matmul(
                        ps[:, 0:VB],
                        lhsT=xt[:, kt, tt * 128:(tt + 1) * 128],
                        rhs=wv_sb[:, kt, vb * VB:(vb + 1) * VB],
                        start=(kt == 0), stop=(kt == N_KT - 1))
                nc.vector.tensor_add(
                    vpad[:, tt, 4 * vb:4 * vb + 4, 0:HD],
                    ps[:, 0:VB].rearrange("p (g d) -> p g d", g=4),
                    bv_sb[:, vb * VB:(vb + 1) * VB].rearrange(
                        "p (g d) -> p g d", g=4))
                T["tensor"] += 9 * MM288
                log_ev("V", (vb, tt))

            def repack_ready(h, qb):
                """Estimated time the head's repack DMAs can have finished."""
                t = 0.0
                for mt in _head_tiles(h):
                    for q in range(N_QB):
                        t = max(t, q_done.get((9 + mt, q), 0.0))
                    t = max(t, q_done.get((mt, qb), 0.0))
                return t + REPACK_NS

            def emit_unit_s(h, qb):
                T["tensor"] = max(T["tensor"], repack_ready(h, qb))
                pieces = _head_pieces(h)
                ktp = pad.tile([128, N], bf16, tag="ktp", bufs=2)
                for (mt, lo, hi, dst) in pieces:
                    nc.sync.dma_start(ktp[dst:dst + hi - lo, :],
                                      qkt[lo:hi, 9 + mt, :])
                qtp = pad.tile([128, QB], bf16, tag="qtp", bufs=4)
                for (mt, lo, hi, dst) in pieces:
                    nc.gpsimd.dma_start(
                        qtp[dst:dst + hi - lo, :],
                        qkt[lo:hi, mt, qb * QB:(qb + 1) * QB])
                ets = []
                for kp in range(N_TT // 2):
                    ps = s_ps.tile([128, 2, QB], f32, tag="s")
                    for j in range(2):
                        kt = 2 * kp + j
                        nc.tensor.matmul(
                            ps[:, j, :],
                            lhsT=ktp[0:HD, kt * 128:(kt + 1) * 128],
                            rhs=qtp[0:HD, :],
                            start=True, stop=True)
                    es = es_pool.tile([128, 2, QB], bf16, tag="e")
                    nc.scalar.activation(es, ps, func=Exp, scale=scale)
                    ets.append(es)
                    T["tensor"] += 2 * MM512
                    T["scalar"] = max(T["scalar"], T["tensor"]) + EXP_T
                es_tiles[(h, qb)] = ets
                exp_done[(h, qb)] = T["scalar"]
                log_ev("S", (h, qb))

            def emit_unit_av(h, qb):
                pieces = _head_pieces(h)
                ets = es_tiles.pop((h, qb))
                ops = ov_ps.tile([128, QB], f32, tag="ov")
                for kt in range(N_TT):
                    nc.tensor.matmul(
                        ops[0:HD + 1, :],
                        lhsT=vpad[:, kt, h, :],
                        rhs=ets[kt // 2][:, kt % 2, :],
                        start=(kt == 0), stop=(kt == N_TT - 1))
                rt = r_pool.tile([96, 2 * QB], f32, tag="r")
                nc.vector.stream_shuffle(
                    rt[0:32, 0:QB], ops[64:96, :], mask=[8] * 32)
                nc.vector.reciprocal_approx_fast(
                    rt[0:32, QB:2 * QB], rt[0:32, 0:QB])
                ident = list(range(32))
                nc.vector.stream_shuffle(
                    rt[32:64, QB:2 * QB], rt[0:32, QB:2 * QB], mask=ident)
                nc.vector.stream_shuffle(
                    rt[64:96, QB:2 * QB], rt[0:32, QB:2 * QB], mask=ident)
                otp = pad.tile([128, QB], bf16, tag="otp", bufs=4)
                nc.vector.tensor_mul(
                    otp[0:HD, :], ops[0:HD, :], rt[0:HD, QB:2 * QB])
                cdma = nc.sync if qb == 0 else nc.gpsimd
                for (mt, lo, hi, dst) in pieces:
                    cdma.dma_start(
                        ot[lo:hi, mt, qb * QB:(qb + 1) * QB],
                        otp[dst:dst + hi - lo, :])
                T["tensor"] = max(T["tensor"] + 8 * MM512,
                                  exp_done[(h, qb)] + 2 * MM512)
                log_ev("AV", (h, qb))

            out_tiles = {}

            def emit_proj_eb(tt, eb):
                """One proj (token-tile, out-block): 9 matmuls (1.47us)."""
                outs = out_pool.tile([128, EB], f32, tag="out")
                ps = qp_ps.tile([128, QB], f32, tag="qp")
                for dt in range(N_KT):
                    nc.tensor.matmul(
                        ps[:, 0:EB],
                        lhsT=ot[:, dt, tt * 128:(tt + 1) * 128],
                        rhs=wp_sb[:, dt, eb * EB:(eb + 1) * EB],
                        start=(dt == 0), stop=(dt == N_KT - 1))
                nc.vector.tensor_add(
                    outs, ps[:, 0:EB], bproj_sb[:, eb * EB:(eb + 1) * EB])
                nc.sync.dma_start(
                    out_d[tt * 128:(tt + 1) * 128, eb * EB:(eb + 1) * EB],
                    outs)
                T["tensor"] += 9 * MM384
                log_ev("proj", (tt, eb))

            # ---------------- schedule ----------------
            # unit order: all qb0 units (h ascending), then all qb1
            units = [(h, 0) for h in range(HEADS)] + \
                    [(h, 1) for h in range(HEADS)]
            s_idx = 0
            av_idx = 0
            # fine-grained filler state
            qk_left = []            # (m, qb) quarters in preferred order
            for j in range(9):
                for m in (9 + j, j):
                    for qb in range(N_QB):
                        qk_left.append((m, qb))
            qk_emitted = set()
            w_issued = {9, 0, 10, 1, 11, 2}
            v_left = [(vb, tt) for vb in range(N_VB) for tt in range(N_TT)]
            v_done_tt = [0] * N_VB
            # proj quarters: gate 0 = qb0 attended, 1 = qb1 attended,
            # 2 = all AV emitted (reserved tail filler for the last-unit
            # normalize/ot-write latency)
            RESERVE = [(2, 0), (2, 1), (2, 2), (3, 0), (3, 1), (3, 2)]
            proj_list = \
                [(tt, eb, 0) for tt in range(4) for eb in range(N_EB)
                 if (tt, eb) not in RESERVE] + \
                [(tt, eb, 2) for (tt, eb) in RESERVE] + \
                [(tt, eb, 1) for tt in range(4, 8) for eb in range(N_EB)]
            proj_done = set()
            qb_done_av = [0, 0]
            wproj_issued = False
            wv_issued = 0

            def s_needs(u):
                """Unemitted qk quarters needed by S(h, qb)."""
                h, qb = u
                need = []
                for mt in _head_tiles(h):
                    for q in range(N_QB):       # ktp reads both halves
                        if (9 + mt, q) not in qk_emitted:
                            need.append((9 + mt, q))
                    if (mt, qb) not in qk_emitted:
                        need.append((mt, qb))
                return need

            def av_ready(u):
                return v_done_tt[u[0] // 4] == N_TT

            def prefetch_w(m, qb):
                """Keep weight DMAs ~2 m-tiles ahead of use (sync queue)."""
                if qb != 1:
                    return
                base = 9 if m >= 9 else 0
                nm = m + 2
                if base <= nm < base + 9 and nm not in w_issued:
                    issue_w(nm, False, nc.sync)
                    w_issued.add(nm)
                    W_RDY[nm] = T["tensor"] + 6000.0

            def do_qk(m, qb):
                if m not in w_issued:
                    issue_w(m, False, nc.sync)
                    w_issued.add(m)
                    W_RDY[m] = T["tensor"] + 6000.0
                emit_qk_quarter(m, qb)
                qk_emitted.add((m, qb))
                qk_left.remove((m, qb))
                prefetch_w(m, qb)

            def do_v():
                vb, tt = v_left.pop(0)
                emit_v_tt(vb, tt)
                v_done_tt[vb] += 1

            def do_av(u, forced=False):
                if forced:
                    T["tensor"] = max(T["tensor"], exp_done[u])
                emit_unit_av(*u)
                qb_done_av[u[1]] += 1

            while s_idx < 32 or av_idx < 32 or qk_left or v_left \
                    or len(proj_done) < len(proj_list):
                # Deferred weight loads: queue position cannot delay a DMA
                # (the scheduler hoists ready instructions), so gate each
                # load with a marker copy whose SOURCE is a late-written qkt
                # m-tile -- the WAW dep on the marker element holds the DMA
                # until that m-tile's eviction really happened.
                if wv_issued < N_VB and s_idx >= 3 * wv_issued and \
                        (0, 0) in qk_emitted:
                    vb = wv_issued
                    gate_m = {0: 0, 1: 11, 2: 13, 3: 15}[vb]
                    nc.vector.tensor_copy(
                        wv_sb[0:1, 0:1, vb * VB:vb * VB + 1],
                        qkt[0:1, gate_m, 0:1])
                    nc.gpsimd.dma_start(
                        wv_sb[:, :, vb * VB:(vb + 1) * VB],
                        wv_r[:, :, vb * VB:(vb + 1) * VB])
                    wv_issued += 1
                if not wproj_issued and s_idx >= 14:
                    for c in range(3):
                        nc.vector.tensor_copy(
                            wp_sb[0:1, 3 * c:3 * c + 1, 0:1],
                            qkt[0:1, 17, 0:1])
                        nc.gpsimd.dma_start(
                            wp_sb[:, 3 * c:3 * c + 3, :],
                            wproj_d[:, 3 * c:3 * c + 3, :])
                    wproj_issued = True

                # 1. AV whose exp is done and V block complete
                if av_idx < s_idx:
                    u = units[av_idx]
                    if av_ready(u) and exp_done[u] <= T["tensor"] + 300:
                        do_av(u)
                        av_idx += 1
                        continue
                # 2. S paced by scalar backlog and repack latency
                if s_idx < 32 and s_idx - av_idx < 2:
                    u = units[s_idx]
                    if not s_needs(u) and \
                            T["scalar"] - T["tensor"] < 2200 and \
                            repack_ready(*u) <= T["tensor"] + 400:
                        emit_unit_s(*u)
                        s_idx += 1
                        continue
                # 3. fillers, finest-urgency first
                # 3a. qk quarters needed by the next S unit
                if s_idx < 32:
                    need = s_needs(units[s_idx])
                    if need:
                        m, qb = need[0]
                        if W_RDY.get(m, 0.0) <= T["tensor"] + 2500:
                            do_qk(m, qb)
                            continue
                # 3b. V tiles for the next blocked AV
                if av_idx < s_idx and not av_ready(units[av_idx]) and \
                        v_left and T["tensor"] >= WV_RDY[v_left[0][0]] - 4000:
                    do_v()
                    continue
                # 3c. proj blocks whose gate is satisfied
                if len(proj_done) < len(proj_list) and wproj_issued:
                    hit = None
                    for (tt, eb, gate) in proj_list:
                        if (tt, eb) in proj_done:
                            continue
                        if gate == 2:
                            ok = av_idx == 32
                        else:
                            ok = qb_done_av[gate] == HEADS
                        if ok:
                            hit = (tt, eb)
                            break
                    if hit is not None:
                        emit_proj_eb(*hit)
                        proj_done.add(hit)
                        continue
                # 3d. spare qk quarter
                if qk_left:
                    for (m, qb) in qk_left:
                        if W_RDY.get(m, 0.0) <= T["tensor"] + 2500:
                            do_qk(m, qb)
                            break
                    else:
                        m, qb = qk_left[0]
                        T["tensor"] = max(T["tensor"], W_RDY.get(m, 0.0))
                        do_qk(m, qb)
                    continue
                # 3e. spare V tile
                if v_left and T["tensor"] >= WV_RDY[v_left[0][0]] - 4000:
                    do_v()
                    continue
                # 4. forced (stall)
                if av_idx < s_idx and av_ready(units[av_idx]):
                    do_av(units[av_idx], forced=True)
                    av_idx += 1
                    continue
                if s_idx < 32 and not s_needs(units[s_idx]):
                    emit_unit_s(*units[s_idx])
                    s_idx += 1
                    continue
                if v_left:
                    T["tensor"] = max(T["tensor"], WV_RDY[v_left[0][0]])
                    do_v()
                    continue
                # last resort: S with repack stall
                if s_idx < 32 and not s_needs(units[s_idx]):
                    u = units[s_idx]
                    T["tensor"] = max(T["tensor"], repack_ready(*u))
                    emit_unit_s(*u)
                    s_idx += 1
                    continue
                raise RuntimeError("emitter deadlock")

            if verbose:
                for (tt_, ts_, kind, tag) in timeline:
                    print(f"  t={tt_/1000:7.2f}us scalar={ts_/1000:7.2f} "
                          f"{kind:5s} {tag}")
                print(f"  predicted tensor end: {T['tensor']/1000:.1f}us")

    nc.compile()
    return nc


def _get_nc():
    if "nc" not in _CACHE:
        _CACHE["nc"] = _build(verbose=False)
    return _CACHE["nc"]


def _prep_shared(Wqkv, bqkv, Wproj, bproj):
    Wqkv = np.asarray(Wqkv, dtype=np.float32)
    bqkv = np.asarray(bqkv, dtype=np.float32)
    Wproj = np.asarray(Wproj, dtype=np.float32)
    bproj = np.asarray(bproj, dtype=np.float32)

    wqk = np.ascontiguousarray(
        Wqkv[:, :QKDIM].reshape(N_KT, 128, N_MT_QK, 128).transpose(2, 1, 0, 3)
    ).astype(ml_dtypes.bfloat16)
    wv = np.ascontiguousarray(Wqkv[:, QKDIM:]).astype(ml_dtypes.bfloat16)
    bqk = np.ascontiguousarray(bqkv[:QKDIM].reshape(N_MT_QK, 128).T)
    bv = np.ascontiguousarray(bqkv[QKDIM:].reshape(1, DIM))
    wproj = np.ascontiguousarray(
        Wproj.reshape(N_KT, 128, DIM).transpose(1, 0, 2)).astype(
            ml_dtypes.bfloat16)
    bproj2 = np.ascontiguousarray(bproj.reshape(1, DIM))
    return dict(wqk=wqk, wv=wv, bqk=bqk, bv=bv, wproj=wproj, bproj=bproj2)


def kernel(x, Wqkv, bqkv, Wproj, bproj, _trace=False):
    from concourse import bass_utils

    x = np.asarray(x, dtype=np.float32).astype(ml_dtypes.bfloat16)
    xt = x.transpose(0, 2, 1).reshape(NCORES, N_KT, 128, N).transpose(
        0, 2, 1, 3)
    shared = _prep_shared(Wqkv, bqkv, Wproj, bproj)
    in_maps = [dict(x=np.ascontiguousarray(xt[i]), **shared)
               for i in range(NCORES)]
    nc = _get_nc()
    res = bass_utils.run_bass_kernel_spmd(
        nc, in_maps, core_ids=list(range(NCORES)), trace=_trace)
    out = np.stack([res.results[i]["out"] for i in range(NCORES)], axis=0)
    if _trace:
        _CACHE["last_exec_time_ns"] = res.exec_time_ns
        _CACHE["last_results"] = res
    return out


if __name__ == "__main__":
    _build(verbose=True)


# revision 4
# speedup vs baseline: 1.0249x; 1.0116x over previous
"""Multi-head attention (B=8, N=1024, DIM=1152, H=16, hd=72) on 8 TRN2 cores.

Sharding: pure data parallelism -- core i computes batch element i, weights
replicated, no collectives.

v2 over the phase-structured baseline: one globally software-pipelined
emission schedule built by a greedy virtual-clock emitter.

  - Startup: x and the first QK weight tiles stream on separate DMA queues
    (sync / scalar) with the first-needed chunks issued first, so the first
    matmul starts at ~10us instead of ~21us.
  - QK m-tiles are emitted in K/Q pairs (m9+j, mj) on demand; attention
    units (per head, per 512-token q-block) start as soon as their two
    stacked-QK m-tiles exist -- S/exp/AV of early heads overlap the rest of
    the QKV projection.
  - exp on ScalarE is the per-unit bottleneck (4.5us vs 3.44us of S+AV
    tensor work), so the emitter paces S-units by a virtual scalar clock
    and fills the gaps with QK pairs / V blocks / proj tiles; leftover
    fillers are spent where attention alone would stall on exp.
  - QKV psum evictions move to DVE (tensor_scalar_add with the per-partition
    bias column) leaving ScalarE exp-only.
  - proj(tt 0..3) runs as filler during qb1 attention; only proj(tt 4..7)
    trails the last attention unit.

Numerics identical to baseline: bf16 matmuls, fp32 psum, exp without max
subtraction (scores ~N(0,1)), softmax denominator via ones-column in the
AV matmul, reciprocal broadcast via stream_shuffle quadrants.
"""

import sys

sys.path.insert(0, "/opt/trn_rl_repo")

import numpy as np
import ml_dtypes

B, N, DIM, HEADS = 8, 1024, 1152, 16
HD = DIM // HEADS  # 72
NCORES = 8
QKDIM = 2 * DIM  # 2304
N_MT_QK = QKDIM // 128  # 18
N_KT = DIM // 128  # 9
N_TT = N // 128  # 8
QB = 512
N_QB = N // QB  # 2
VB = 288  # v block = 4 heads
N_VB = DIM // VB  # 4
EB = 384
N_EB = DIM // EB  # 3

_CACHE = {}

# virtual-clock cost estimates (ns)
MM512 = 215
MM288 = 121
MM384 = 163
EXP_T = 1125
S_UNIT = 8 * MM512
AV_UNIT = 8 * MM512


def _head_pieces(h):
    """Pieces covering rows [72h, 72h+72) of a 128-row-tiled stack."""
    r0 = HD * h
    mt, p0 = divmod(r0, 128)
    ln = min(HD, 128 - p0)
    pieces = [(mt, p0, p0 + ln, 0)]
    if ln < HD:
        pieces.append((mt + 1, 0, HD - ln, ln))
    return pieces


def _head_tiles(h):
    return sorted({mt for (mt, _, _, _) in _head_pieces(h)})


def _build(verbose=False):
    import concourse.tile as tile
    from concourse import bacc, mybir

    f32 = mybir.dt.float32
    bf16 = mybir.dt.bfloat16
    Exp = mybir.ActivationFunctionType.Exp

    nc = bacc.Bacc("TRN2", target_bir_lowering=False, debug=False,
                   num_devices=NCORES)

    x_d = nc.dram_tensor("x", [128, N_KT, N], bf16,
                         kind="ExternalInput").ap()
    wqk_d = nc.dram_tensor("wqk", [N_MT_QK, 128, N_KT, 128], bf16,
                           kind="ExternalInput").ap()
    wv_d = nc.dram_tensor("wv", [DIM, DIM], bf16, kind="ExternalInput").ap()
    bqk_d = nc.dram_tensor("bqk", [128, N_MT_QK], f32,
                           kind="ExternalInput").ap()
    bv_d = nc.dram_tensor("bv", [1, DIM], f32, kind="ExternalInput").ap()
    wproj_d = nc.dram_tensor("wproj", [128, N_KT, DIM], bf16,
                             kind="ExternalInput").ap()
    bproj_d = nc.dram_tensor("bproj", [1, DIM], f32,
                             kind="ExternalInput").ap()
    out_d = nc.dram_tensor("out", [N, DIM], f32, kind="ExternalOutput").ap()

    scale = float(HD) ** -0.5

    with tile.TileContext(nc) as tc:
        with tc.tile_pool(name="consts", bufs=1) as consts, \
             tc.tile_pool(name="persist", bufs=1) as persist, \
             tc.tile_pool(name="xt_pool", bufs=1) as xt_pool, \
             tc.tile_pool(name="wv_pool", bufs=1) as wv_pool, \
             tc.tile_pool(name="wqk_pool", bufs=8) as wqk_pool, \
             tc.tile_pool(name="wp_pool", bufs=1) as wp_pool, \
             tc.tile_pool(name="pad", bufs=1) as pad, \
             tc.tile_pool(name="es_pool", bufs=12) as es_pool, \
             tc.tile_pool(name="r_pool", bufs=2) as r_pool, \
             tc.tile_pool(name="out_pool", bufs=2) as out_pool, \
             tc.tile_pool(name="qp_ps", bufs=2, space="PSUM") as qp_ps, \
             tc.tile_pool(name="s_ps", bufs=2, space="PSUM") as s_ps, \
             tc.tile_pool(name="ov_ps", bufs=2, space="PSUM") as ov_ps:

            # ---------------- prologue DMAs ----------------
            # x on sync: qb0 chunks first (first QK pair needs them)
            xt = xt_pool.tile([128, N_KT, N], bf16)
            for qb in range(N_QB):
                nc.sync.dma_start(xt[:, 0:3, qb * QB:(qb + 1) * QB],
                                  x_d[:, 0:3, qb * QB:(qb + 1) * QB])
                nc.sync.dma_start(xt[:, 3:9, qb * QB:(qb + 1) * QB],
                                  x_d[:, 3:9, qb * QB:(qb + 1) * QB])
            bqk_sb = consts.tile([128, N_MT_QK], f32)
            nc.sync.dma_start(bqk_sb, bqk_d)

            # first QK weight tiles on the scalar queue (idle until exp)
            w_tiles = {}

            def issue_w(m, chunked, eng):
                w_t = wqk_pool.tile([128, N_KT, 128], bf16, tag="w")
                if chunked:
                    eng.dma_start(w_t[:, 0:3, :], wqk_d[m, :, 0:3, :])
                    eng.dma_start(w_t[:, 3:9, :], wqk_d[m, :, 3:9, :])
                else:
                    eng.dma_start(w_t, wqk_d[m])
                w_tiles[m] = w_t

            issue_w(9, False, nc.scalar)
            issue_w(0, False, nc.scalar)
            issue_w(10, False, nc.scalar)
            issue_w(1, False, nc.scalar)
            issue_w(11, False, nc.scalar)
            issue_w(2, False, nc.scalar)

            # biases + wv on gpsimd
            bv_sb = consts.tile([128, DIM], f32)
            nc.gpsimd.dma_start(bv_sb[0:1, :], bv_d)
            nc.gpsimd.partition_broadcast(bv_sb, bv_sb[0:1, :], channels=128)
            bproj_sb = consts.tile([128, DIM], f32)
            nc.gpsimd.dma_start(bproj_sb[0:1, :], bproj_d)
            nc.gpsimd.partition_broadcast(bproj_sb, bproj_sb[0:1, :],
                                          channels=128)
            # wv split by v-block; every block marker-gated off a qkt
            # eviction so none of it crowds the x/wqk startup window
            wv_sb = wv_pool.tile([128, N_KT, DIM], bf16)
            wv_r = wv_d.rearrange("(kt kp) v -> kp kt v", kp=128)

            # persistent activations
            qkt = persist.tile([128, N_MT_QK, N], bf16)
            vpad = persist.tile([128, N_TT, HEADS, HD + 1], bf16)
            nc.vector.memset(vpad[:, :, :, HD:HD + 1], 1.0)
            ot = persist.tile([128, N_KT, N], bf16)
            wp_sb = wp_pool.tile([128, N_KT, DIM], bf16)

            # ---------------- virtual-clock emitter ----------------
            T = {"tensor": 10000.0, "scalar": 10000.0}
            exp_done = {}
            es_tiles = {}
            # availability estimates (ns), W_RDY keyed by m-tile
            X_RDY = {0: 14000.0, 1: 20000.0}  # full x for q-block
            W_RDY = {9: 12500.0, 0: 14000.0, 10: 17000.0, 1: 18500.0,
                     11: 20500.0, 2: 22000.0}
            WV_RDY = [30000.0, 60000.0, 80000.0, 100000.0]  # per vb
            REPACK_NS = 5500.0  # evict -> sbuf repack DMA latency
            q_done = {}
            timeline = []

            def log_ev(kind, tag):
                timeline.append((T["tensor"], T["scalar"], kind, tag))

            def emit_qk_mtile(m, qb):
                T["tensor"] = max(T["tensor"], X_RDY[qb])
                ps = qp_ps.tile([128, QB], f32, tag="qp")
                w_t = w_tiles[m]
                for kt in range(N_KT):
                    nc.tensor.matmul(
                        ps, lhsT=w_t[:, kt, :],
                        rhs=xt[:, kt, qb * QB:(qb + 1) * QB],
                        start=(kt == 0), stop=(kt == N_KT - 1))
                nc.vector.tensor_scalar_add(
                    qkt[:, m, qb * QB:(qb + 1) * QB], ps, bqk_sb[:, m:m + 1])
                T["tensor"] += 9 * MM512

            def emit_qk_quarter(m, qb):
                """One (m-tile, q-block) = 9 matmuls + DVE evict (1.94us)."""
                T["tensor"] = max(T["tensor"], X_RDY[qb],
                                  W_RDY.get(m, 0.0))
                emit_qk_mtile(m, qb)
                q_done[(m, qb)] = T["tensor"]
                log_ev("qk", (m, qb))

            def emit_v_tt(vb, tt):
                """One V token-tile of a 4-head block (1.09us)."""
                T["tensor"] = max(T["tensor"], WV_RDY[vb], X_RDY[1])
                ps = ov_ps.tile([128, QB], f32, tag="ov")
                for kt in range(N_KT):
                    nc.tensor.matmul(
                        ps[:, 0:VB],
                        lhsT=xt[:, kt, tt * 128:(tt + 1) * 128],
                        rhs=wv_sb[:, kt, vb * VB:(vb + 1) * VB],
                        start=(kt == 0), stop=(kt == N_KT - 1))
                nc.vector.tensor_add(
                    vpad[:, tt, 4 * vb:4 * vb + 4, 0:HD],
                    ps[:, 0:VB].rearrange("p (g d) -> p g d", g=4),
                    bv_sb[:, vb * VB:(vb + 1) * VB].rearrange(
                        "p (g d) -> p g d", g=4))
                T["tensor"] += 9 * MM288
                log_ev("V", (vb, tt))

            def repack_ready(h, qb):
                """Estimated time the head's repack DMAs can have finished."""
                t = 0.0
                for mt in _head_tiles(h):
                    for q in range(N_QB):
                        t = max(t, q_done.get((9 + mt, q), 0.0))
                    t = max(t, q_done.get((mt, qb), 0.0))
                return t + REPACK_NS

            def emit_unit_s(h, qb):
                T["tensor"] = max(T["tensor"], repack_ready(h, qb))
                pieces = _head_pieces(h)
                ktp = pad.tile([128, N], bf16, tag="ktp", bufs=2)
                for (mt, lo, hi, dst) in pieces:
                    nc.sync.dma_start(ktp[dst:dst + hi - lo, :],
                                      qkt[lo:hi, 9 + mt, :])
                qtp = pad.tile([128, QB], bf16, tag="qtp", bufs=4)
                for (mt, lo, hi, dst) in pieces:
                    nc.gpsimd.dma_start(
                        qtp[dst:dst + hi - lo, :],
                        qkt[lo:hi, mt, qb * QB:(qb + 1) * QB])
                ets = []
                for kp in range(N_TT // 2):
                    ps = s_ps.tile([128, 2, QB], f32, tag="s")
                    for j in range(2):
                        kt = 2 * kp + j
                        nc.tensor.matmul(
                            ps[:, j, :],
                            lhsT=ktp[0:HD, kt * 128:(kt + 1) * 128],
                            rhs=qtp[0:HD, :],
                            start=True, stop=True)
                    es = es_pool.tile([128, 2, QB], bf16, tag="e")
                    nc.scalar.activation(es, ps, func=Exp, scale=scale)
                    ets.append(es)
                    T["tensor"] += 2 * MM512
                    T["scalar"] = max(T["scalar"], T["tensor"]) + EXP_T
                es_tiles[(h, qb)] = ets
                exp_done[(h, qb)] = T["scalar"]
                log_ev("S", (h, qb))

            def emit_unit_av(h, qb):
                pieces = _head_pieces(h)
                ets = es_tiles.pop((h, qb))
                ops = ov_ps.tile([128, QB], f32, tag="ov")
                for kt in range(N_TT):
                    nc.tensor.matmul(
                        ops[0:HD + 1, :],
                        lhsT=vpad[:, kt, h, :],
                        rhs=ets[kt // 2][:, kt % 2, :],
                        start=(kt == 0), stop=(kt == N_TT - 1))
                rt = r_pool.tile([96, 2 * QB], f32, tag="r")
                nc.vector.stream_shuffle(
                    rt[0:32, 0:QB], ops[64:96, :], mask=[8] * 32)
                nc.vector.reciprocal_approx_fast(
                    rt[0:32, QB:2 * QB], rt[0:32, 0:QB])
                ident = list(range(32))
                nc.vector.stream_shuffle(
                    rt[32:64, QB:2 * QB], rt[0:32, QB:2 * QB], mask=ident)
                nc.vector.stream_shuffle(
                    rt[64:96, QB:2 * QB], rt[0:32, QB:2 * QB], mask=ident)
                otp = pad.tile([128, QB], bf16, tag="otp", bufs=4)
                nc.vector.tensor_mul(
                    otp[0:HD, :], ops[0:HD, :], rt[0:HD, QB:2 * QB])
                cdma = nc.sync if qb == 0 else nc.gpsimd
                for (mt, lo, hi, dst) in pieces:
                    cdma.dma_start(
                        ot[lo:hi, mt, qb * QB:(qb + 1) * QB],
                        otp[dst:dst + hi - lo, :])
                T["tensor"] = max(T["tensor"] + 8 * MM512,
                                  exp_done[(h, qb)] + 2 * MM512)
                log_ev("AV", (h, qb))

            out_tiles = {}

            def emit_proj_eb(tt, eb):
                """One proj (token-tile, out-block): 9 matmuls (1.47us)."""
                outs = out_pool.tile([128, EB], f32, tag="out")
                ps = qp_ps.tile([128, QB], f32, tag="qp")
                for dt in range(N_KT):
                    nc.tensor.matmul(
                        ps[:, 0:EB],
                        lhsT=ot[:, dt, tt * 128:(tt + 1) * 128],
                        rhs=wp_sb[:, dt, eb * EB:(eb + 1) * EB],
                        start=(dt == 0), stop=(dt == N_KT - 1))
                nc.vector.tensor_add(
                    outs, ps[:, 0:EB], bproj_sb[:, eb * EB:(eb + 1) * EB])
                nc.sync.dma_start(
                    out_d[tt * 128:(tt + 1) * 128, eb * EB:(eb + 1) * EB],
                    outs)
                T["tensor"] += 9 * MM384
                log_ev("proj", (tt, eb))

            # ---------------- schedule ----------------
            # unit order: all qb0 units (h ascending), then all qb1
            units = [(h, 0) for h in range(HEADS)] + \
                    [(h, 1) for h in range(HEADS)]
            s_idx = 0
            av_idx = 0
            # fine-grained filler state
            qk_left = []            # (m, qb) quarters in preferred order
            # qb-major within each pair: both qb0 quarters run before any
            # qb1 work, so the first pair never waits on the late x qb1
            for j in range(9):
                for qb in range(N_QB):
                    for m in (9 + j, j):
                        qk_left.append((m, qb))
            qk_emitted = set()
            w_issued = {9, 0, 10, 1, 11, 2}
            v_left = [(vb, tt) for vb in range(N_VB) for tt in range(N_TT)]
            v_done_tt = [0] * N_VB
            # proj quarters: gate 0 = qb0 attended, 1 = qb1 attended,
            # 2 = all AV emitted (reserved tail filler for the last-unit
            # normalize/ot-write latency)
            RESERVE = [(2, 0), (2, 1), (2, 2), (3, 0), (3, 1), (3, 2)]
            proj_list = \
                [(tt, eb, 0) for tt in range(4) for eb in range(N_EB)
                 if (tt, eb) not in RESERVE] + \
                [(tt, eb, 2) for (tt, eb) in RESERVE] + \
                [(tt, eb, 1) for tt in range(4, 8) for eb in range(N_EB)]
            proj_done = set()
            qb_done_av = [0, 0]
            wproj_issued = False
            wv_issued = 0

            def s_needs(u):
                """Unemitted qk quarters needed by S(h, qb)."""
                h, qb = u
                need = []
                for mt in _head_tiles(h):
                    for q in range(N_QB):       # ktp reads both halves
                        if (9 + mt, q) not in qk_emitted:
                            need.append((9 + mt, q))
                    if (mt, qb) not in qk_emitted:
                        need.append((mt, qb))
                return need

            def av_ready(u):
                return v_done_tt[u[0] // 4] == N_TT

            def prefetch_w(m, qb):
                """Keep weight DMAs ~2 m-tiles ahead of use (sync queue)."""
                if qb != 1:
                    return
                base = 9 if m >= 9 else 0
                nm = m + 2
                if base <= nm < base + 9 and nm not in w_issued:
                    issue_w(nm, False, nc.sync)
                    w_issued.add(nm)
                    W_RDY[nm] = T["tensor"] + 6000.0

            def do_qk(m, qb):
                if m not in w_issued:
                    issue_w(m, False, nc.sync)
                    w_issued.add(m)
                    W_RDY[m] = T["tensor"] + 6000.0
                emit_qk_quarter(m, qb)
                qk_emitted.add((m, qb))
                qk_left.remove((m, qb))
                prefetch_w(m, qb)

            def do_v():
                vb, tt = v_left.pop(0)
                emit_v_tt(vb, tt)
                v_done_tt[vb] += 1

            def do_av(u, forced=False):
                if forced:
                    T["tensor"] = max(T["tensor"], exp_done[u])
                emit_unit_av(*u)
                qb_done_av[u[1]] += 1

            while s_idx < 32 or av_idx < 32 or qk_left or v_left \
                    or len(proj_done) < len(proj_list):
                # Deferred weight loads: queue position cannot delay a DMA
                # (the scheduler hoists ready instructions), so gate each
                # load with a marker copy whose SOURCE is a late-written qkt
                # m-tile -- the WAW dep on the marker element holds the DMA
                # until that m-tile's eviction really happened.
                if wv_issued < N_VB and s_idx >= 3 * wv_issued and \
                        (0, 0) in qk_emitted:
                    vb = wv_issued
                    gate_m = {0: 0, 1: 11, 2: 13, 3: 15}[vb]
                    nc.vector.tensor_copy(
                        wv_sb[0:1, 0:1, vb * VB:vb * VB + 1],
                        qkt[0:1, gate_m, 0:1])
                    nc.gpsimd.dma_start(
                        wv_sb[:, :, vb * VB:(vb + 1) * VB],
                        wv_r[:, :, vb * VB:(vb + 1) * VB])
                    wv_issued += 1
                if not wproj_issued and s_idx >= 14:
                    for c in range(3):
                        nc.vector.tensor_copy(
                            wp_sb[0:1, 3 * c:3 * c + 1, 0:1],
                            qkt[0:1, 17, 0:1])
                        nc.gpsimd.dma_start(
                            wp_sb[:, 3 * c:3 * c + 3, :],
                            wproj_d[:, 3 * c:3 * c + 3, :])
                    wproj_issued = True

                # 1. AV whose exp is done and V block complete
                if av_idx < s_idx:
                    u = units[av_idx]
                    if av_ready(u) and exp_done[u] <= T["tensor"] + 300:
                        do_av(u)
                        av_idx += 1
                        continue
                # 2. S paced by scalar backlog and repack latency
                if s_idx < 32 and s_idx - av_idx < 2:
                    u = units[s_idx]
                    if not s_needs(u) and \
                            T["scalar"] - T["tensor"] < 2200 and \
                            repack_ready(*u) <= T["tensor"] + 400:
                        emit_unit_s(*u)
                        s_idx += 1
                        continue
                # 3. fillers, finest-urgency first
                # 3a. qk quarters needed by the next S unit
                if s_idx < 32:
                    need = s_needs(units[s_idx])
                    if need:
                        m, qb = need[0]
                        if W_RDY.get(m, 0.0) <= T["tensor"] + 2500:
                            do_qk(m, qb)
                            continue
                # 3b. V tiles for the next blocked AV
                if av_idx < s_idx and not av_ready(units[av_idx]) and \
                        v_left and T["tensor"] >= WV_RDY[v_left[0][0]] - 4000:
                    do_v()
                    continue
                # 3c. proj blocks whose gate is satisfied
                if len(proj_done) < len(proj_list) and wproj_issued:
                    hit = None
                    for (tt, eb, gate) in proj_list:
                        if (tt, eb) in proj_done:
                            continue
                        if gate == 2:
                            ok = av_idx == 32
                        else:
                            ok = qb_done_av[gate] == HEADS
                        if ok:
                            hit = (tt, eb)
                            break
                    if hit is not None:
                        emit_proj_eb(*hit)
                        proj_done.add(hit)
                        continue
                # 3d. spare qk quarter
                if qk_left:
                    for (m, qb) in qk_left:
                        if W_RDY.get(m, 0.0) <= T["tensor"] + 2500:
                            do_qk(m, qb)
                            break
                    else:
                        m, qb = qk_left[0]
                        T["tensor"] = max(T["tensor"], W_RDY.get(m, 0.0))
                        do_qk(m, qb)
                    continue
                # 3e. spare V tile
                if v_left and T["tensor"] >= WV_RDY[v_left[0][0]] - 4000:
                    do_v()
                    continue
                # 4. forced (stall)
                if av_idx < s_idx and av_ready(units[av_idx]):
                    do_av(units[av_idx], forced=True)
                    av_idx += 1
                    continue
                if s_idx < 32 and not s_needs(units[s_idx]):
                    emit_unit_s(*units[s_idx])
                    s_idx += 1
                    continue
                if v_left:
                    T["tensor"] = max(T["tensor"], WV_RDY[v_left[0][0]])
                    do_v()
                    continue
                # last resort: S with repack stall
                if s_idx < 32 and not s_needs(units[s_idx]):
                    u = units[s_idx]
                    T["tensor"] = max(T["tensor"], repack_ready(*u))
                    emit_unit_s(*u)
                    s_idx += 1
                    continue
                raise RuntimeError("emitter deadlock")

            if verbose:
                for (tt_, ts_, kind, tag) in timeline:
                    print(f"  t={tt_/1000:7.2f}us scalar={ts_/1000:7.2f} "
                          f"{kind:5s} {tag}")
                print(f"  predicted tensor end: {T['tensor']/1000:.1f}us")

    nc.compile()
    return nc


def _get_nc():
    if "nc" not in _CACHE:
        _CACHE["nc"] = _build(verbose=False)
    return _CACHE["nc"]


def _prep_shared(Wqkv, bqkv, Wproj, bproj):
    Wqkv = np.asarray(Wqkv, dtype=np.float32)
    bqkv = np.asarray(bqkv, dtype=np.float32)
    Wproj = np.asarray(Wproj, dtype=np.float32)
    bproj = np.asarray(bproj, dtype=np.float32)

    wqk = np.ascontiguousarray(
        Wqkv[:, :QKDIM].reshape(N_KT, 128, N_MT_QK, 128).transpose(2, 1, 0, 3)
    ).astype(ml_dtypes.bfloat16)
    wv = np.ascontiguousarray(Wqkv[:, QKDIM:]).astype(ml_dtypes.bfloat16)
    bqk = np.ascontiguousarray(bqkv[:QKDIM].reshape(N_MT_QK, 128).T)
    bv = np.ascontiguousarray(bqkv[QKDIM:].reshape(1, DIM))
    wproj = np.ascontiguousarray(
        Wproj.reshape(N_KT, 128, DIM).transpose(1, 0, 2)).astype(
            ml_dtypes.bfloat16)
    bproj2 = np.ascontiguousarray(bproj.reshape(1, DIM))
    return dict(wqk=wqk, wv=wv, bqk=bqk, bv=bv, wproj=wproj, bproj=bproj2)


def kernel(x, Wqkv, bqkv, Wproj, bproj, _trace=False):
    from concourse import bass_utils

    x = np.asarray(x, dtype=np.float32).astype(ml_dtypes.bfloat16)
    xt = x.transpose(0, 2, 1).reshape(NCORES, N_KT, 128, N).transpose(
        0, 2, 1, 3)
    shared = _prep_shared(Wqkv, bqkv, Wproj, bproj)
    in_maps = [dict(x=np.ascontiguousarray(xt[i]), **shared)
               for i in range(NCORES)]
    nc = _get_nc()
    res = bass_utils.run_bass_kernel_spmd(
        nc, in_maps, core_ids=list(range(NCORES)), trace=_trace)
    out = np.stack([res.results[i]["out"] for i in range(NCORES)], axis=0)
    if _trace:
        _CACHE["last_exec_time_ns"] = res.exec_time_ns
        _CACHE["last_results"] = res
    return out


if __name__ == "__main__":
    _build(verbose=True)
